# revision 18
# baseline (speedup 1.0000x reference)
"""AttnDecoderRNN single-step kernel for one TRN2 chip (8 NeuronCores).

Sharding strategy (tensor-parallel over 8 cores):
  - attention core (tiny) replicated on every core (fp32)
  - comb projection: output dim H sharded 128/core (bf16 weights)
  - GRU: contraction-sharded — core k computes partial gate pre-activations
    for ALL 3072 gate outputs from its x/h slice (bf16 weights); one 16KB
    AllReduce(add) of [pre_r | pre_z | gi_n | gh_n] (bias folded into core 0's
    contribution) gives every core the full gates -> full h_new locally.
    This replaces two AllGathers with a single collective.
  - out projection: vocab padded to 51200, sharded 6400/core, bf16 weights
    kept stationary on the PE; logits land vocab-on-partitions [128, 50].
  - log-softmax: fused exp+row-sum on ACT, partition-sum via ones-matmul,
    AllGather of the 8 partial sums, subtract log(S).

DMA ring split: the 12.8MB out-projection weight stream issues on the sync
(SP) HWDGE ring; all small latency-critical DMAs issue on the scalar (ACT)
ring so they are not FIFO-blocked behind the stream.

All DRAM-facing layouts are contiguous; the host de-interleaves outputs.
"""

import os
import sys

import numpy as np

try:
    import concourse.bass as bass  # noqa: F401
except ImportError:  # pragma: no cover - fallback when PYTHONPATH is not set
    for _p in (
        "/root/.axon_site",
        "/root/.axon_site/_ro/trn_rl_repo",
        "/root/.axon_site/_ro/pypackages",
        "/opt/trn_rl_repo",
    ):
        if os.path.isdir(_p) and _p not in sys.path:
            sys.path.insert(0, _p)
    import concourse.bass as bass  # noqa: F401

import ml_dtypes
import concourse.bacc as bacc
import concourse.tile as tile
from concourse import mybir
from concourse.bass_utils import run_bass_kernel_spmd
from concourse.tile import add_dep_helper

H = 1024
V = 50257
L = 10
NCORES = 8
HC = 8          # chunks of 128 along H
VPAD = 51200    # vocab padded to 8*6400
VS = VPAD // NCORES   # per-core vocab shard = 6400
VT = VS // 128        # v-tiles per core = 50
TPG = 5               # v-tiles per DMA group
G = VT // TPG         # weight-stream DMA groups = 10
NEG_BIG = -30000.0    # bias pad so exp() of padded logits underflows to 0

F32 = mybir.dt.float32
BF16 = mybir.dt.bfloat16
FP8 = mybir.dt.float8e3      # e3m4: 4 mantissa bits, max ~15.5
NP_FP8 = ml_dtypes.float8_e3m4
W_SCALE = 64.0               # out_W quantization scale; logits rescaled by 1/64
AF = mybir.ActivationFunctionType
OP = mybir.AluOpType

# packF32 [128, NF] layout offsets
_oAW, _oAB2 = 0, None
_oATTN = 0            # attnWT 16*L = 160
_oARB = 160           # arB (gate bias, core0 only) 32
_oCB = 192            # combB 1
_oEMB = 193           # embT 8
_oHT = 201            # hT 8
_oHK = 209            # hk 1
_oBIAS = 210          # biasT VT=50
NF = 260
# packC [128, 2048] bf16: combWT; packG [128, NB] bf16: wih|whh col-sharded
_oWIH = 0             # 24*128 = 3072
_oWHH = 3072
NB = 6144
N_EARLY = 3           # wout groups streamed before the AllReduce; rest gated on it

_compiled = None        # cached nc built once per process
_last_results = None    # BassKernelResults of the most recent hardware run


def _ensure_ntff_hook():
    """bass_utils' trace path imports antenv.axon_hooks, which this image
    lacks; register an equivalent shim backed by the boot module's ctypes
    NTFF driver so trace=True / BASS_TRACE=1 works instead of crashing."""
    import importlib.util
    import types

    try:
        if importlib.util.find_spec("antenv.axon_hooks") is not None:
            return
    except ModuleNotFoundError:
        pass
    mod = types.ModuleType("antenv.axon_hooks")
    state = {"hook": None}
    mod.set_axon_ntff_profile_hook = lambda h: state.__setitem__("hook", h)
    mod.get_axon_ntff_profile_hook = lambda: state["hook"]
    sys.modules["antenv.axon_hooks"] = mod
    try:
        from trn_agent_boot.trn_boot import _ntff_profile_via_ctypes

        hook = _ntff_profile_via_ctypes("/opt/axon/libaxon_pjrt.so")
        if hook is not None:
            state["hook"] = hook
    except Exception:
        pass


_ensure_ntff_hook()


def _emit(tc, I, O):
    """Emit the SPMD per-core program. I/O are dicts of DRAM APs."""
    nc = tc.nc
    grp = [list(range(NCORES))]

    with (
        tc.tile_pool(name="sp", bufs=1) as sp,
        tc.tile_pool(name="wo", bufs=G) as wo,
        tc.tile_pool(name="ps", bufs=4, space="PSUM") as psp,
        tc.tile_pool(name="pa", bufs=1, space="PSUM") as psa,
        tc.tile_pool(name="po", bufs=1, space="PSUM") as pso,
        tc.tile_pool(name="dp", bufs=1, space="DRAM") as dp,
    ):
        # ---- weight stream on the sync ring, critical-path tensors first ----
        # (HWDGE is FIFO per ring: packF/encP/packB must precede the big
        # out-projection stream; mid-kernel DMAs go on the scalar ring.)
        with tc.high_priority():
            packF = sp.tile([128, NF], F32)
            nc.sync.dma_start(out=packF[:], in_=I["packF"])
            encP = sp.tile([L, H + 1], F32)
            nc.sync.dma_start(out=encP[:], in_=I["encP"])
            packC = sp.tile([128, 2048], BF16)
            nc.sync.dma_start(out=packC[:], in_=I["packC"])
            packG = sp.tile([128, NB], BF16)
            nc.sync.dma_start(out=packG[:], in_=I["packG"])
        wg = []
        wout_late = []
        for g in range(G):
            wt = wo.tile([128, TPG, HC, 128], FP8, tag="wt")
            dma = nc.sync.dma_start(out=wt[:], in_=I["wout"][g])
            if g >= N_EARLY:
                wout_late.append(dma)
            wg.append(wt)

        attnWT = packF[:, _oATTN : _oATTN + 16 * L].rearrange(
            "p (c m) -> p c m", c=16
        )
        arB = packF[:, _oARB : _oARB + 32]
        combB = packF[:, _oCB : _oCB + 1]
        embT = packF[:, _oEMB : _oEMB + HC]
        hT = packF[:, _oHT : _oHT + HC]
        hk = packF[:, _oHK : _oHK + 1]
        biasT = packF[:, _oBIAS : _oBIAS + VT]
        enc = encP[:, 0:H]
        attnB = encP[:, H : H + 1]
        combWT = packC[:].rearrange("p (c m) -> p c m", c=16)
        wihV = packG[:, _oWIH : _oWIH + 3072].rearrange("p (t m) -> p t m", t=24)
        whhV = packG[:, _oWHH : _oWHH + 3072].rearrange("p (t m) -> p t m", t=24)

        ones = sp.tile([128, 1], F32)
        nc.vector.memset(ones[:], 1.0)
        onesr = sp.tile([1, 128], F32)
        nc.vector.memset(onesr[:], 1.0)
        # bf16 copies of the moving vectors
        embB = sp.tile([128, HC], BF16)
        nc.vector.tensor_copy(embB[:], embT)
        hkB = sp.tile([128, 1], BF16)
        nc.vector.tensor_copy(hkB[:], hk)

        # ---- attention (replicated, fp32) ----
        ps_a = psp.tile([L, 1], F32, tag="ps")
        for c in range(16):
            xin = embT[:, c : c + 1] if c < HC else hT[:, c - HC : c - HC + 1]
            nc.tensor.matmul(
                ps_a[:], attnWT[:, c, :], xin, start=(c == 0), stop=(c == 15)
            )
        e_a = sp.tile([L, 1], F32)
        nc.scalar.activation(e_a[:], ps_a[:], AF.Exp, bias=attnB, scale=1.0)
        # warm the Ln table now so the log-softmax tail doesn't pay the
        # ~1.3us ACT_TABLE_LOAD on the critical path
        lnwarm = sp.tile([1, 1], F32)
        nc.scalar.activation(lnwarm[:], ones[0:1, :], AF.Ln)
        ps_s = psp.tile([1, 1], F32, tag="ps")
        nc.tensor.matmul(ps_s[:], e_a[:], ones[0:L, :], start=True, stop=True)
        rec = sp.tile([1, 1], F32)
        nc.vector.reciprocal(rec[:], ps_s[:])
        ps_r = psp.tile([128, 1], F32, tag="ps")
        nc.tensor.matmul(ps_r[:], onesr[:], rec[:], start=True, stop=True)
        rbc = sp.tile([128, 1], F32)
        nc.vector.tensor_copy(rbc[:], ps_r[:])
        aw = sp.tile([L, 1], F32)
        nc.vector.tensor_scalar_mul(aw[:], in0=e_a[:], scalar1=rbc[0:L, :])
        nc.scalar.dma_start(out=O["out_aw"], in_=aw[:])
        ps_att = psp.tile([128, HC], F32, tag="ps")
        for m in range(HC):
            nc.tensor.matmul(
                ps_att[:, m : m + 1],
                enc[:, m * 128 : (m + 1) * 128],
                e_a[:],
                start=True,
                stop=True,
            )
        attB = sp.tile([128, HC], BF16)
        nc.vector.tensor_scalar_mul(attB[:], in0=ps_att[:], scalar1=rbc[:])

        # ---- comb projection (sharded output slice) + relu, bf16 ----
        ps_c = psp.tile([128, 1], F32, tag="ps")
        for c in range(16):
            xin = embB[:, c : c + 1] if c < HC else attB[:, c - HC : c - HC + 1]
            nc.tensor.matmul(
                ps_c[:], combWT[:, c, :], xin, start=(c == 0), stop=(c == 15)
            )
        xkB = sp.tile([128, 1], BF16)
        nc.scalar.activation(xkB[:], ps_c[:], AF.Relu, bias=combB, scale=1.0)

        # ---- GRU partial gates from local x/h slices (no gather needed) ----
        # ps_acc cols: 0-7 pre_r, 8-15 pre_z, 16-23 gi_n, 24-31 gh_n
        ps_acc = psa.tile([128, 32], F32)
        for mt in range(16):
            nc.tensor.matmul(
                ps_acc[:, mt : mt + 1], wihV[:, mt, :], xkB[:], start=True, stop=False
            )
            nc.tensor.matmul(
                ps_acc[:, mt : mt + 1], whhV[:, mt, :], hkB[:], start=False, stop=True
            )
        for j in range(8):
            nc.tensor.matmul(
                ps_acc[:, 16 + j : 17 + j], wihV[:, 16 + j, :], xkB[:],
                start=True, stop=True,
            )
        for j in range(8):
            nc.tensor.matmul(
                ps_acc[:, 24 + j : 25 + j], whhV[:, 16 + j, :], hkB[:],
                start=True, stop=True,
            )
        # add bias (nonzero only on core 0 so the AllReduce adds it once)
        arin = sp.tile([128, 32], F32)
        nc.vector.tensor_add(out=arin[:], in0=ps_acc[:], in1=arB)

        # ---- AllReduce the partial gates ----
        ar_in = dp.tile([4096], F32)
        ar_out = dp.tile([4096], F32)
        nc.scalar.dma_start(
            out=ar_in.rearrange("(p c) -> p c", p=128), in_=arin[:]
        )
        cc_ar = nc.gpsimd.collective_compute(
            "AllReduce", OP.add, replica_groups=grp,
            ins=[ar_in.opt()], outs=[ar_out.opt()],
        )
        # hold back the bulk of the weight stream until the AllReduce is done:
        # collectives' control plane is latency-bound and degrades ~4x when the
        # HBM/SDMA path is saturated by the stream
        for dma in wout_late:
            add_dep_helper(dma.ins, cc_ar.ins, sync=True,
                           reason="late wout groups wait for AR (quiet HBM)")
        hg = sp.tile([128, 32], F32)
        nc.scalar.dma_start(
            out=hg[:], in_=ar_out.rearrange("(p c) -> p c", p=128)
        )

        # ---- gates -> full h_new (all [128, HC] chunk layout) ----
        # sigmoid/tanh via exp so ACT never switches tables:
        #   sigmoid(x) = 1/(1+exp(-x)); tanh(x) = 2/(1+exp(-2x)) - 1
        erz = sp.tile([128, 16], F32)
        nc.scalar.activation(erz[:], hg[:, 0:16], AF.Exp, scale=-1.0)
        erz1 = sp.tile([128, 16], F32)
        nc.vector.tensor_scalar_add(erz1[:], in0=erz[:], scalar1=1.0)
        rz = sp.tile([128, 16], F32)
        nc.vector.reciprocal(rz[:], erz1[:])
        t2 = sp.tile([128, HC], F32)
        nc.vector.tensor_mul(out=t2[:], in0=rz[:, 0:8], in1=hg[:, 24:32])
        t3 = sp.tile([128, HC], F32)
        nc.vector.tensor_add(out=t3[:], in0=t2[:], in1=hg[:, 16:24])
        en = sp.tile([128, HC], F32)
        nc.scalar.activation(en[:], t3[:], AF.Exp, scale=-2.0)
        en1 = sp.tile([128, HC], F32)
        nc.vector.tensor_scalar_add(en1[:], in0=en[:], scalar1=1.0)
        rn = sp.tile([128, HC], F32)
        nc.vector.reciprocal(rn[:], en1[:])
        n_g = sp.tile([128, HC], F32)
        nc.vector.tensor_scalar(
            out=n_g[:], in0=rn[:], scalar1=2.0, scalar2=-1.0,
            op0=OP.mult, op1=OP.add,
        )
        d_g = sp.tile([128, HC], F32)
        nc.vector.tensor_sub(out=d_g[:], in0=hT, in1=n_g[:])
        t4 = sp.tile([128, HC], F32)
        nc.vector.tensor_mul(out=t4[:], in0=rz[:, 8:16], in1=d_g[:])
        hnT = sp.tile([128, HC], F32)
        nc.vector.tensor_add(out=hnT[:], in0=t4[:], in1=n_g[:])
        nc.scalar.dma_start(out=O["out_h"], in_=hnT[:])
        hnb = sp.tile([128, HC], FP8)
        nc.vector.tensor_copy(hnb[:], hnT[:])

        # ---- out projection: logitsT [128, VT], vocab on partitions ----
        ps_o = pso.tile([128, VT], F32)
        for t in range(VT):
            g, tt = divmod(t, TPG)
            for c in range(HC):
                nc.tensor.matmul(
                    ps_o[:, t : t + 1],
                    wg[g][:, tt, c, :],
                    hnb[:, c : c + 1],
                    start=(c == 0),
                    stop=(c == HC - 1),
                )
        # logits = psum/W_SCALE + bias  (undo the fp8 weight quantization scale)
        logits = sp.tile([128, VT], F32)
        nc.vector.scalar_tensor_tensor(
            out=logits[:], in0=ps_o[:], scalar=1.0 / W_SCALE, in1=biasT,
            op0=OP.mult, op1=OP.add,
        )

        # ---- distributed log-softmax ----
        e_o = sp.tile([128, VT], F32)
        sums = sp.tile([128, 1], F32)
        nc.scalar.activation(e_o[:], logits[:], AF.Exp, accum_out=sums[:])
        ps_t = psp.tile([1, 1], F32, tag="ps")
        nc.tensor.matmul(ps_t[:], sums[:], ones[:], start=True, stop=True)
        spad = sp.tile([1, 8], F32)
        nc.vector.memset(spad[:], 0.0)
        nc.vector.tensor_copy(spad[:, 0:1], ps_t[:])
        ags_in = dp.tile([8], F32)
        ags_out = dp.tile([64], F32)
        nc.scalar.dma_start(out=ags_in[:], in_=spad[:])
        nc.gpsimd.collective_compute(
            "AllGather", OP.bypass, replica_groups=grp,
            ins=[ags_in.opt()], outs=[ags_out.opt()],
        )
        s8 = sp.tile([1, 64], F32)
        nc.scalar.dma_start(out=s8[:], in_=ags_out[:])
        tot = sp.tile([1, 1], F32)
        nc.vector.tensor_reduce(tot[:], s8[:], axis=mybir.AxisListType.X, op=OP.add)
        lns = sp.tile([1, 1], F32)
        nc.scalar.activation(lns[:], tot[:], AF.Ln)
        ps_l = psp.tile([128, 1], F32, tag="ps")
        nc.tensor.matmul(ps_l[:], onesr[:], lns[:], start=True, stop=True)
        lnb = sp.tile([128, 1], F32)
        nc.vector.tensor_copy(lnb[:], ps_l[:])
        out_sb = sp.tile([128, VT], F32)
        nc.vector.tensor_scalar(
            out=out_sb[:], in0=logits[:], scalar1=lnb[:], scalar2=None, op0=OP.subtract
        )
        nc.scalar.dma_start(out=O["out_lp"], in_=out_sb[:])


def _build():
    nc = bacc.Bacc(
        "TRN2", target_bir_lowering=False, debug=False, num_devices=NCORES
    )

    def inp(name, shape, dt=F32):
        return nc.dram_tensor(name, shape, dt, kind="ExternalInput").ap()

    def outp(name, shape, dt=F32):
        return nc.dram_tensor(name, shape, dt, kind="ExternalOutput").ap()

    I = dict(
        packF=inp("packF", [128, NF]),
        encP=inp("encP", [L, H + 1]),
        packC=inp("packC", [128, 2048], BF16),
        packG=inp("packG", [128, NB], BF16),
        wout=inp("wout", [G, 128, TPG, HC, 128], FP8),
    )
    O = dict(
        out_lp=outp("out_lp", [128, VT]),
        out_h=outp("out_h", [128, HC]),
        out_aw=outp("out_aw", [L]),
    )
    with tile.TileContext(nc) as tc:
        _emit(tc, I, O)
    nc.compile()
    return nc


def get_compiled():
    global _compiled
    if _compiled is None:
        _compiled = _build()
    return _compiled


def _chunkT(vec):
    """[1024] -> [128, 8] where out[p, c] = vec[c*128 + p]."""
    return np.ascontiguousarray(vec.reshape(HC, 128).T)


def _prepare_in_maps(inputs):
    f32 = np.float32
    inp = {k: np.asarray(v) for k, v in inputs.items()}
    tok = int(np.asarray(inp["input"]).ravel()[0])
    emb_row = np.ascontiguousarray(inp["emb"][tok], dtype=f32)       # [H]
    h = np.ascontiguousarray(inp["hidden"], dtype=f32).reshape(H)    # [H]
    enc = np.ascontiguousarray(inp["encoder_outputs"], dtype=f32)    # [L,H]
    attn_W = np.asarray(inp["attn_W"], f32)      # [L, 2H]
    attn_b = np.asarray(inp["attn_b"], f32)      # [L]
    comb_W = np.asarray(inp["comb_W"], f32)      # [H, 2H]
    comb_b = np.asarray(inp["comb_b"], f32)      # [H]
    wih = np.asarray(inp["gru_wih"], f32)        # [3H, H]
    whh = np.asarray(inp["gru_whh"], f32)        # [3H, H]
    bih = np.asarray(inp["gru_bih"], f32)        # [3H]
    bhh = np.asarray(inp["gru_bhh"], f32)        # [3H]
    out_W = np.asarray(inp["out_W"], f32)        # [V, H]
    out_b = np.asarray(inp["out_b"], f32)        # [V]

    embT = _chunkT(emb_row)
    hT = _chunkT(h)
    attnWT = np.ascontiguousarray(
        attn_W.T.reshape(16, 128, L).transpose(1, 0, 2)
    ).reshape(128, 16 * L)
    encP = np.concatenate([enc, attn_b.reshape(L, 1)], axis=1)       # [L, H+1]

    bsum = bih + bhh
    arB0 = np.concatenate(
        [
            _chunkT(bsum[0:H]),
            _chunkT(bsum[H : 2 * H]),
            _chunkT(bih[2 * H :]),
            _chunkT(bhh[2 * H :]),
        ],
        axis=1,
    )                                                                # [128, 32]

    W_pad = np.zeros((VPAD, H), dtype=f32)
    W_pad[:V] = out_W
    b_pad = np.full((VPAD,), NEG_BIG, dtype=f32)
    b_pad[:V] = out_b

    in_maps = []
    for k in range(NCORES):
        sl = slice(k * 128, (k + 1) * 128)
        combWT = (
            comb_W[sl].T.reshape(16, 128, 128).transpose(1, 0, 2).reshape(128, 2048)
        )
        wihC = wih[:, sl].T.reshape(128, 3072)   # [p, t*128+m] col-sharded
        whhC = whh[:, sl].T.reshape(128, 3072)

        packF = np.zeros((128, NF), dtype=f32)
        packF[:, _oATTN : _oATTN + 16 * L] = attnWT
        if k == 0:
            packF[:, _oARB : _oARB + 32] = arB0
        packF[:, _oCB] = comb_b[sl]
        packF[:, _oEMB : _oEMB + HC] = embT
        packF[:, _oHT : _oHT + HC] = hT
        packF[:, _oHK] = h[sl]
        packF[:, _oBIAS : _oBIAS + VT] = b_pad[k * VS : (k + 1) * VS].reshape(VT, 128).T

        packC = np.ascontiguousarray(combWT.astype(ml_dtypes.bfloat16))
        packG = np.zeros((128, NB), dtype=ml_dtypes.bfloat16)
        packG[:, _oWIH : _oWIH + 3072] = wihC.astype(ml_dtypes.bfloat16)
        packG[:, _oWHH : _oWHH + 3072] = whhC.astype(ml_dtypes.bfloat16)

        Wk = W_pad[k * VS : (k + 1) * VS] * W_SCALE                  # [VS, H]
        A = Wk.reshape(VT, 128, HC, 128).transpose(0, 3, 2, 1)       # [t,p,c,m]
        wout = np.ascontiguousarray(
            A.reshape(G, TPG, 128, HC, 128).transpose(0, 2, 1, 3, 4)
        ).astype(NP_FP8)                                             # [G,128,TPG,HC,128]

        in_maps.append(
            dict(packF=packF, encP=encP, packC=packC, packG=packG, wout=wout)
        )
    return in_maps


def _assemble(results):
    # out_lp [128, VT] with element [p, t] = log_prob[k*VS + t*128 + p]
    lp = np.concatenate(
        [np.asarray(results[k]["out_lp"]).T.ravel() for k in range(NCORES)]
    )
    log_probs = lp[:V].reshape(1, V).astype(np.float32)
    hidden = (
        np.asarray(results[0]["out_h"]).T.ravel().reshape(1, 1, H).astype(np.float32)
    )
    attn_w = np.asarray(results[0]["out_aw"]).reshape(1, L).astype(np.float32)
    return log_probs, hidden, attn_w


def kernel(**inputs):
    global _last_results
    nc = get_compiled()
    in_maps = _prepare_in_maps(inputs)

    if os.environ.get("KERNEL_SIM"):
        from concourse.bass_interp import MultiCoreSim

        sim = MultiCoreSim(nc, num_cores=NCORES)
        for i in range(NCORES):
            for k, v in in_maps[i].items():
                sim.cores[i].tensor(k)[:] = v
        sim.simulate()
        results = [
            {n: np.array(sim.cores[i].tensor(n)) for n in ("out_lp", "out_h", "out_aw")}
            for i in range(NCORES)
        ]
        return _assemble(results)

    res = run_bass_kernel_spmd(nc, in_maps, list(range(NCORES)))
    _last_results = res
    return _assemble(res.results)


# revision 20
# speedup vs baseline: 1.2222x; 1.2222x over previous
"""AttnDecoderRNN single-step kernel for one TRN2 chip (8 NeuronCores).

Sharding strategy (tensor-parallel over 8 cores):
  - attention core (tiny) replicated on every core (fp32)
  - comb projection: output dim H sharded 128/core (bf16 weights)
  - GRU: contraction-sharded — core k computes partial gate pre-activations
    for ALL 3072 gate outputs from its x/h slice (bf16 weights); one 16KB
    AllReduce(add) of [pre_r | pre_z | gi_n | gh_n] (bias folded into core 0's
    contribution) gives every core the full gates -> full h_new locally.
    This replaces two AllGathers with a single collective.
  - out projection: vocab padded to 51200, sharded 6400/core, bf16 weights
    kept stationary on the PE; logits land vocab-on-partitions [128, 50].
  - log-softmax: fused exp+row-sum on ACT, partition-sum via ones-matmul,
    AllGather of the 8 partial sums, subtract log(S).

DMA ring split: the 12.8MB out-projection weight stream issues on the sync
(SP) HWDGE ring; all small latency-critical DMAs issue on the scalar (ACT)
ring so they are not FIFO-blocked behind the stream.

All DRAM-facing layouts are contiguous; the host de-interleaves outputs.
"""

import os
import sys

import numpy as np

try:
    import concourse.bass as bass  # noqa: F401
except ImportError:  # pragma: no cover - fallback when PYTHONPATH is not set
    for _p in (
        "/root/.axon_site",
        "/root/.axon_site/_ro/trn_rl_repo",
        "/root/.axon_site/_ro/pypackages",
        "/opt/trn_rl_repo",
    ):
        if os.path.isdir(_p) and _p not in sys.path:
            sys.path.insert(0, _p)
    import concourse.bass as bass  # noqa: F401

import ml_dtypes
import concourse.bacc as bacc
import concourse.tile as tile
from concourse import mybir
from concourse.bass_utils import run_bass_kernel_spmd
from concourse.tile import add_dep_helper

H = 1024
V = 50257
L = 10
NCORES = 8
HC = 8          # chunks of 128 along H
VPAD = 51200    # vocab padded to 8*6400
VS = VPAD // NCORES   # per-core vocab shard = 6400
VT = VS // 128        # v-tiles per core = 50
TPG = 5               # v-tiles per DMA group
G = VT // TPG         # weight-stream DMA groups = 10
NEG_BIG = -30000.0    # bias pad so exp() of padded logits underflows to 0

F32 = mybir.dt.float32
BF16 = mybir.dt.bfloat16
FP8 = mybir.dt.float8e3      # e3m4: 4 mantissa bits, max ~15.5
NP_FP8 = ml_dtypes.float8_e3m4
W_SCALE = 64.0               # out_W quantization scale; logits rescaled by 1/64
AF = mybir.ActivationFunctionType
OP = mybir.AluOpType

# packF32 [128, NF] layout offsets
_oAW, _oAB2 = 0, None
_oATTN = 0            # attnWT 16*L = 160
_oARB = 160           # arB (gate bias, core0 only) 32
_oCB = 192            # combB 1
_oEMB = 193           # embT 8
_oHT = 201            # hT 8
_oHK = 209            # hk 1
_oBIAS = 210          # biasT VT=50
NF = 260
# packC [128, 2048] bf16: combWT; packG [128, NB] bf16: wih|whh col-sharded
_oWIH = 0             # 24*128 = 3072
_oWHH = 3072
NB = 6144
N_EARLY = 3           # wout groups streamed before the AllReduce; rest gated on it

_compiled = None        # cached nc built once per process
_last_results = None    # BassKernelResults of the most recent hardware run


def _ensure_ntff_hook():
    """bass_utils' trace path imports antenv.axon_hooks, which this image
    lacks; register an equivalent shim backed by the boot module's ctypes
    NTFF driver so trace=True / BASS_TRACE=1 works instead of crashing."""
    import importlib.util
    import types

    try:
        if importlib.util.find_spec("antenv.axon_hooks") is not None:
            return
    except ModuleNotFoundError:
        pass
    mod = types.ModuleType("antenv.axon_hooks")
    state = {"hook": None}
    mod.set_axon_ntff_profile_hook = lambda h: state.__setitem__("hook", h)
    mod.get_axon_ntff_profile_hook = lambda: state["hook"]
    sys.modules["antenv.axon_hooks"] = mod
    try:
        from trn_agent_boot.trn_boot import _ntff_profile_via_ctypes

        hook = _ntff_profile_via_ctypes("/opt/axon/libaxon_pjrt.so")
        if hook is not None:
            state["hook"] = hook
    except Exception:
        pass


_ensure_ntff_hook()


def _emit(tc, I, O):
    """Emit the SPMD per-core program. I/O are dicts of DRAM APs."""
    nc = tc.nc
    grp = [list(range(NCORES))]

    with (
        tc.tile_pool(name="sp", bufs=1) as sp,
        tc.tile_pool(name="wo", bufs=G) as wo,
        tc.tile_pool(name="ps", bufs=4, space="PSUM") as psp,
        tc.tile_pool(name="pa", bufs=1, space="PSUM") as psa,
        tc.tile_pool(name="po", bufs=1, space="PSUM") as pso,
        tc.tile_pool(name="dp", bufs=1, space="DRAM") as dp,
    ):
        # ---- weight stream on the sync ring, critical-path tensors first ----
        # (HWDGE is FIFO per ring: packF/encP/packB must precede the big
        # out-projection stream; mid-kernel DMAs go on the scalar ring.)
        with tc.high_priority():
            packF = sp.tile([128, NF], F32)
            nc.sync.dma_start(out=packF[:], in_=I["packF"])
            encP = sp.tile([L, H + 1], F32)
            nc.sync.dma_start(out=encP[:], in_=I["encP"])
            packC = sp.tile([128, 2048], BF16)
            nc.sync.dma_start(out=packC[:], in_=I["packC"])
            packG = sp.tile([128, NB], BF16)
            nc.sync.dma_start(out=packG[:], in_=I["packG"])
        wg = []
        for g in range(G):
            wt = wo.tile([128, TPG, HC, 128], FP8, tag="wt")
            nc.sync.dma_start(out=wt[:], in_=I["wout"][g])
            wg.append(wt)

        attnWT = packF[:, _oATTN : _oATTN + 16 * L].rearrange(
            "p (c m) -> p c m", c=16
        )
        arB = packF[:, _oARB : _oARB + 32]
        combB = packF[:, _oCB : _oCB + 1]
        embT = packF[:, _oEMB : _oEMB + HC]
        hT = packF[:, _oHT : _oHT + HC]
        hk = packF[:, _oHK : _oHK + 1]
        biasT = packF[:, _oBIAS : _oBIAS + VT]
        enc = encP[:, 0:H]
        attnB = encP[:, H : H + 1]
        combWT = packC[:].rearrange("p (c m) -> p c m", c=16)
        wihV = packG[:, _oWIH : _oWIH + 3072].rearrange("p (t m) -> p t m", t=24)
        whhV = packG[:, _oWHH : _oWHH + 3072].rearrange("p (t m) -> p t m", t=24)

        ones = sp.tile([128, 1], F32)
        nc.vector.memset(ones[:], 1.0)
        onesr = sp.tile([1, 128], F32)
        nc.vector.memset(onesr[:], 1.0)
        # bf16 copies of the moving vectors
        embB = sp.tile([128, HC], BF16)
        nc.vector.tensor_copy(embB[:], embT)
        hkB = sp.tile([128, 1], BF16)
        nc.vector.tensor_copy(hkB[:], hk)

        # ---- attention (replicated, fp32) ----
        ps_a = psp.tile([L, 1], F32, tag="ps")
        for c in range(16):
            xin = embT[:, c : c + 1] if c < HC else hT[:, c - HC : c - HC + 1]
            nc.tensor.matmul(
                ps_a[:], attnWT[:, c, :], xin, start=(c == 0), stop=(c == 15)
            )
        e_a = sp.tile([L, 1], F32)
        nc.scalar.activation(e_a[:], ps_a[:], AF.Exp, bias=attnB, scale=1.0)
        # warm the Ln table now so the log-softmax tail doesn't pay the
        # ~1.3us ACT_TABLE_LOAD on the critical path
        lnwarm = sp.tile([1, 1], F32)
        nc.scalar.activation(lnwarm[:], ones[0:1, :], AF.Ln)
        ps_s = psp.tile([1, 1], F32, tag="ps")
        nc.tensor.matmul(ps_s[:], e_a[:], ones[0:L, :], start=True, stop=True)
        rec = sp.tile([1, 1], F32)
        nc.vector.reciprocal(rec[:], ps_s[:])
        ps_r = psp.tile([128, 1], F32, tag="ps")
        nc.tensor.matmul(ps_r[:], onesr[:], rec[:], start=True, stop=True)
        rbc = sp.tile([128, 1], F32)
        nc.vector.tensor_copy(rbc[:], ps_r[:])
        aw = sp.tile([L, 1], F32)
        nc.vector.tensor_scalar_mul(aw[:], in0=e_a[:], scalar1=rbc[0:L, :])
        nc.scalar.dma_start(out=O["out_aw"], in_=aw[:])
        ps_att = psp.tile([128, HC], F32, tag="ps")
        for m in range(HC):
            nc.tensor.matmul(
                ps_att[:, m : m + 1],
                enc[:, m * 128 : (m + 1) * 128],
                e_a[:],
                start=True,
                stop=True,
            )
        attB = sp.tile([128, HC], BF16)
        nc.vector.tensor_scalar_mul(attB[:], in0=ps_att[:], scalar1=rbc[:])

        # ---- comb projection (sharded output slice) + relu, bf16 ----
        ps_c = psp.tile([128, 1], F32, tag="ps")
        for c in range(16):
            xin = embB[:, c : c + 1] if c < HC else attB[:, c - HC : c - HC + 1]
            nc.tensor.matmul(
                ps_c[:], combWT[:, c, :], xin, start=(c == 0), stop=(c == 15)
            )
        xkB = sp.tile([128, 1], BF16)
        nc.scalar.activation(xkB[:], ps_c[:], AF.Relu, bias=combB, scale=1.0)

        # ---- GRU partial gates from local x/h slices (no gather needed) ----
        # ps_acc cols: 0-7 pre_r, 8-15 pre_z, 16-23 gi_n, 24-31 gh_n
        ps_acc = psa.tile([128, 32], F32)
        for mt in range(16):
            nc.tensor.matmul(
                ps_acc[:, mt : mt + 1], wihV[:, mt, :], xkB[:], start=True, stop=False
            )
            nc.tensor.matmul(
                ps_acc[:, mt : mt + 1], whhV[:, mt, :], hkB[:], start=False, stop=True
            )
        for j in range(8):
            nc.tensor.matmul(
                ps_acc[:, 16 + j : 17 + j], wihV[:, 16 + j, :], xkB[:],
                start=True, stop=True,
            )
        for j in range(8):
            nc.tensor.matmul(
                ps_acc[:, 24 + j : 25 + j], whhV[:, 16 + j, :], hkB[:],
                start=True, stop=True,
            )
        # add bias (nonzero only on core 0 so the AllReduce adds it once)
        arin = sp.tile([128, 32], F32)
        nc.vector.tensor_add(out=arin[:], in0=ps_acc[:], in1=arB)

        # ---- AllGather the partial gates + local 8-way reduction ----
        # (AllReduce on this stack runs a ~14-step ring at ~38us; AllGather is
        # ~5-8us. Gather all 8 partial blocks and reduce on-core instead.)
        ar_in = dp.tile([4096], F32)
        ar_out = dp.tile([NCORES * 4096], F32)
        nc.scalar.dma_start(
            out=ar_in.rearrange("(p c) -> p c", p=128), in_=arin[:]
        )
        nc.gpsimd.collective_compute(
            "AllGather", OP.bypass, replica_groups=grp,
            ins=[ar_in.opt()], outs=[ar_out.opt()],
        )
        hga = sp.tile([128, NCORES, 32], F32)
        nc.scalar.dma_start(
            out=hga[:], in_=ar_out.rearrange("(k p c) -> p k c", p=128, c=32)
        )
        hg = sp.tile([128, 32], F32)
        nc.vector.tensor_reduce(
            hg[:], hga[:].rearrange("p k c -> p c k"),
            axis=mybir.AxisListType.X, op=OP.add,
        )

        # ---- gates -> full h_new (all [128, HC] chunk layout) ----
        # sigmoid/tanh via exp so ACT never switches tables:
        #   sigmoid(x) = 1/(1+exp(-x)); tanh(x) = 2/(1+exp(-2x)) - 1
        erz = sp.tile([128, 16], F32)
        nc.scalar.activation(erz[:], hg[:, 0:16], AF.Exp, scale=-1.0)
        erz1 = sp.tile([128, 16], F32)
        nc.vector.tensor_scalar_add(erz1[:], in0=erz[:], scalar1=1.0)
        rz = sp.tile([128, 16], F32)
        nc.vector.reciprocal(rz[:], erz1[:])
        t2 = sp.tile([128, HC], F32)
        nc.vector.tensor_mul(out=t2[:], in0=rz[:, 0:8], in1=hg[:, 24:32])
        t3 = sp.tile([128, HC], F32)
        nc.vector.tensor_add(out=t3[:], in0=t2[:], in1=hg[:, 16:24])
        en = sp.tile([128, HC], F32)
        nc.scalar.activation(en[:], t3[:], AF.Exp, scale=-2.0)
        en1 = sp.tile([128, HC], F32)
        nc.vector.tensor_scalar_add(en1[:], in0=en[:], scalar1=1.0)
        rn = sp.tile([128, HC], F32)
        nc.vector.reciprocal(rn[:], en1[:])
        n_g = sp.tile([128, HC], F32)
        nc.vector.tensor_scalar(
            out=n_g[:], in0=rn[:], scalar1=2.0, scalar2=-1.0,
            op0=OP.mult, op1=OP.add,
        )
        d_g = sp.tile([128, HC], F32)
        nc.vector.tensor_sub(out=d_g[:], in0=hT, in1=n_g[:])
        t4 = sp.tile([128, HC], F32)
        nc.vector.tensor_mul(out=t4[:], in0=rz[:, 8:16], in1=d_g[:])
        hnT = sp.tile([128, HC], F32)
        nc.vector.tensor_add(out=hnT[:], in0=t4[:], in1=n_g[:])
        nc.scalar.dma_start(out=O["out_h"], in_=hnT[:])
        hnb = sp.tile([128, HC], FP8)
        nc.vector.tensor_copy(hnb[:], hnT[:])

        # ---- out projection: logitsT [128, VT], vocab on partitions ----
        ps_o = pso.tile([128, VT], F32)
        for t in range(VT):
            g, tt = divmod(t, TPG)
            for c in range(HC):
                nc.tensor.matmul(
                    ps_o[:, t : t + 1],
                    wg[g][:, tt, c, :],
                    hnb[:, c : c + 1],
                    start=(c == 0),
                    stop=(c == HC - 1),
                )
        # logits = psum/W_SCALE + bias  (undo the fp8 weight quantization scale)
        logits = sp.tile([128, VT], F32)
        nc.vector.scalar_tensor_tensor(
            out=logits[:], in0=ps_o[:], scalar=1.0 / W_SCALE, in1=biasT,
            op0=OP.mult, op1=OP.add,
        )

        # ---- distributed log-softmax ----
        e_o = sp.tile([128, VT], F32)
        sums = sp.tile([128, 1], F32)
        nc.scalar.activation(e_o[:], logits[:], AF.Exp, accum_out=sums[:])
        ps_t = psp.tile([1, 1], F32, tag="ps")
        nc.tensor.matmul(ps_t[:], sums[:], ones[:], start=True, stop=True)
        spad = sp.tile([1, 8], F32)
        nc.vector.memset(spad[:], 0.0)
        nc.vector.tensor_copy(spad[:, 0:1], ps_t[:])
        ags_in = dp.tile([8], F32)
        ags_out = dp.tile([64], F32)
        nc.scalar.dma_start(out=ags_in[:], in_=spad[:])
        nc.gpsimd.collective_compute(
            "AllGather", OP.bypass, replica_groups=grp,
            ins=[ags_in.opt()], outs=[ags_out.opt()],
        )
        s8 = sp.tile([1, 64], F32)
        nc.scalar.dma_start(out=s8[:], in_=ags_out[:])
        tot = sp.tile([1, 1], F32)
        nc.vector.tensor_reduce(tot[:], s8[:], axis=mybir.AxisListType.X, op=OP.add)
        lns = sp.tile([1, 1], F32)
        nc.scalar.activation(lns[:], tot[:], AF.Ln)
        ps_l = psp.tile([128, 1], F32, tag="ps")
        nc.tensor.matmul(ps_l[:], onesr[:], lns[:], start=True, stop=True)
        lnb = sp.tile([128, 1], F32)
        nc.vector.tensor_copy(lnb[:], ps_l[:])
        out_sb = sp.tile([128, VT], F32)
        nc.vector.tensor_scalar(
            out=out_sb[:], in0=logits[:], scalar1=lnb[:], scalar2=None, op0=OP.subtract
        )
        nc.scalar.dma_start(out=O["out_lp"], in_=out_sb[:])


def _build():
    nc = bacc.Bacc(
        "TRN2", target_bir_lowering=False, debug=False, num_devices=NCORES
    )

    def inp(name, shape, dt=F32):
        return nc.dram_tensor(name, shape, dt, kind="ExternalInput").ap()

    def outp(name, shape, dt=F32):
        return nc.dram_tensor(name, shape, dt, kind="ExternalOutput").ap()

    I = dict(
        packF=inp("packF", [128, NF]),
        encP=inp("encP", [L, H + 1]),
        packC=inp("packC", [128, 2048], BF16),
        packG=inp("packG", [128, NB], BF16),
        wout=inp("wout", [G, 128, TPG, HC, 128], FP8),
    )
    O = dict(
        out_lp=outp("out_lp", [128, VT]),
        out_h=outp("out_h", [128, HC]),
        out_aw=outp("out_aw", [L]),
    )
    with tile.TileContext(nc) as tc:
        _emit(tc, I, O)
    nc.compile()
    return nc


def get_compiled():
    global _compiled
    if _compiled is None:
        _compiled = _build()
    return _compiled


def _chunkT(vec):
    """[1024] -> [128, 8] where out[p, c] = vec[c*128 + p]."""
    return np.ascontiguousarray(vec.reshape(HC, 128).T)


def _prepare_in_maps(inputs):
    f32 = np.float32
    inp = {k: np.asarray(v) for k, v in inputs.items()}
    tok = int(np.asarray(inp["input"]).ravel()[0])
    emb_row = np.ascontiguousarray(inp["emb"][tok], dtype=f32)       # [H]
    h = np.ascontiguousarray(inp["hidden"], dtype=f32).reshape(H)    # [H]
    enc = np.ascontiguousarray(inp["encoder_outputs"], dtype=f32)    # [L,H]
    attn_W = np.asarray(inp["attn_W"], f32)      # [L, 2H]
    attn_b = np.asarray(inp["attn_b"], f32)      # [L]
    comb_W = np.asarray(inp["comb_W"], f32)      # [H, 2H]
    comb_b = np.asarray(inp["comb_b"], f32)      # [H]
    wih = np.asarray(inp["gru_wih"], f32)        # [3H, H]
    whh = np.asarray(inp["gru_whh"], f32)        # [3H, H]
    bih = np.asarray(inp["gru_bih"], f32)        # [3H]
    bhh = np.asarray(inp["gru_bhh"], f32)        # [3H]
    out_W = np.asarray(inp["out_W"], f32)        # [V, H]
    out_b = np.asarray(inp["out_b"], f32)        # [V]

    embT = _chunkT(emb_row)
    hT = _chunkT(h)
    attnWT = np.ascontiguousarray(
        attn_W.T.reshape(16, 128, L).transpose(1, 0, 2)
    ).reshape(128, 16 * L)
    encP = np.concatenate([enc, attn_b.reshape(L, 1)], axis=1)       # [L, H+1]

    bsum = bih + bhh
    arB0 = np.concatenate(
        [
            _chunkT(bsum[0:H]),
            _chunkT(bsum[H : 2 * H]),
            _chunkT(bih[2 * H :]),
            _chunkT(bhh[2 * H :]),
        ],
        axis=1,
    )                                                                # [128, 32]

    W_pad = np.zeros((VPAD, H), dtype=f32)
    W_pad[:V] = out_W
    b_pad = np.full((VPAD,), NEG_BIG, dtype=f32)
    b_pad[:V] = out_b

    in_maps = []
    for k in range(NCORES):
        sl = slice(k * 128, (k + 1) * 128)
        combWT = (
            comb_W[sl].T.reshape(16, 128, 128).transpose(1, 0, 2).reshape(128, 2048)
        )
        wihC = wih[:, sl].T.reshape(128, 3072)   # [p, t*128+m] col-sharded
        whhC = whh[:, sl].T.reshape(128, 3072)

        packF = np.zeros((128, NF), dtype=f32)
        packF[:, _oATTN : _oATTN + 16 * L] = attnWT
        if k == 0:
            packF[:, _oARB : _oARB + 32] = arB0
        packF[:, _oCB] = comb_b[sl]
        packF[:, _oEMB : _oEMB + HC] = embT
        packF[:, _oHT : _oHT + HC] = hT
        packF[:, _oHK] = h[sl]
        packF[:, _oBIAS : _oBIAS + VT] = b_pad[k * VS : (k + 1) * VS].reshape(VT, 128).T

        packC = np.ascontiguousarray(combWT.astype(ml_dtypes.bfloat16))
        packG = np.zeros((128, NB), dtype=ml_dtypes.bfloat16)
        packG[:, _oWIH : _oWIH + 3072] = wihC.astype(ml_dtypes.bfloat16)
        packG[:, _oWHH : _oWHH + 3072] = whhC.astype(ml_dtypes.bfloat16)

        Wk = W_pad[k * VS : (k + 1) * VS] * W_SCALE                  # [VS, H]
        A = Wk.reshape(VT, 128, HC, 128).transpose(0, 3, 2, 1)       # [t,p,c,m]
        wout = np.ascontiguousarray(
            A.reshape(G, TPG, 128, HC, 128).transpose(0, 2, 1, 3, 4)
        ).astype(NP_FP8)                                             # [G,128,TPG,HC,128]

        in_maps.append(
            dict(packF=packF, encP=encP, packC=packC, packG=packG, wout=wout)
        )
    return in_maps


def _assemble(results):
    # out_lp [128, VT] with element [p, t] = log_prob[k*VS + t*128 + p]
    lp = np.concatenate(
        [np.asarray(results[k]["out_lp"]).T.ravel() for k in range(NCORES)]
    )
    log_probs = lp[:V].reshape(1, V).astype(np.float32)
    hidden = (
        np.asarray(results[0]["out_h"]).T.ravel().reshape(1, 1, H).astype(np.float32)
    )
    attn_w = np.asarray(results[0]["out_aw"]).reshape(1, L).astype(np.float32)
    return log_probs, hidden, attn_w


def kernel(**inputs):
    global _last_results
    nc = get_compiled()
    in_maps = _prepare_in_maps(inputs)

    if os.environ.get("KERNEL_SIM"):
        from concourse.bass_interp import MultiCoreSim

        sim = MultiCoreSim(nc, num_cores=NCORES)
        for i in range(NCORES):
            for k, v in in_maps[i].items():
                sim.cores[i].tensor(k)[:] = v
        sim.simulate()
        results = [
            {n: np.array(sim.cores[i].tensor(n)) for n in ("out_lp", "out_h", "out_aw")}
            for i in range(NCORES)
        ]
        return _assemble(results)

    res = run_bass_kernel_spmd(nc, in_maps, list(range(NCORES)))
    _last_results = res
    return _assemble(res.results)
